# revision 14
# baseline (speedup 1.0000x reference)
"""Grouped-GEMM (MoE routing) kernel for TRN2, 8 NeuronCores, SPMD.

out[m] = values[m] @ combining_matrix[species_idx[m]]
  values [131072, 128] f32, species_idx [131072] i32, combining_matrix [8, 128, 256] f32

Strategy:
  - Host: counting-sort rows by species; deal each species' rows round-robin
    across the 8 cores so per-core per-species counts are balanced (+-1).
    Each core's rows are packed species-contiguous into a transposed buffer
    xT [128, R_pad] (species segment s zero-padded to a static capacity C[s],
    identical on every core -> one SPMD program).
  - Device (per core): keep all 8 weight matrices resident in SBUF
    ([128, 8*256] = 8KB/partition). For each species s and output half
    h in {0,1}: out_T[h*128:(h+1)*128, seg_s] = W[s][:, h*128:+128].T @ xT[:, seg_s]
    via matmuls with 512-column moving chunks (fp32, K=128 contraction on
    partitions). PSUM -> SBUF copy -> DMA to outT [256, R_pad].
  - Host: scatter outT columns back to the full [131072, 256] output.

This does 1x the FLOPs of the reference's 8x masked-matmul formulation and is
DMA-roofline-bound (~27 MB/core HBM traffic).
"""

import numpy as np
from contextlib import ExitStack

import concourse.bass as bass
import concourse.mybir as mybir
import concourse.tile as tile
from concourse import bacc
from concourse.bass_utils import run_bass_kernel_spmd

M_TOTAL = 131072
D_IN = 128
N_OUT = 256
N_SPECIES = 8
N_CORES = 8
PAD = 128          # species segment capacity granularity (rows)
CHUNK = 512        # matmul moving-dim chunk (max for fp32)
F32 = mybir.dt.float32
USE_FP32R = True   # fp32r matmul mode: full PE rate (vs 1/4 for fp32)
MM_DT = mybir.dt.float32r if USE_FP32R else mybir.dt.float32

def _build_nc(caps, r_pad):
    """Build the SPMD program for one core. caps[s] = padded column count of
    species segment s (same on all cores); r_pad = sum(caps)."""
    nc = bacc.Bacc("TRN2", target_bir_lowering=False, debug=False,
                   num_devices=N_CORES)
    xT = nc.dram_tensor("xT", [D_IN, r_pad], MM_DT, kind="ExternalInput").ap()
    w = nc.dram_tensor("w", [D_IN, N_SPECIES * N_OUT], MM_DT,
                       kind="ExternalInput").ap()
    outT = nc.dram_tensor("outT", [N_OUT, r_pad], F32, kind="ExternalOutput").ap()

    seg_off = {}
    off = 0
    for s in range(N_SPECIES):
        seg_off[s] = off
        off += caps[s]

    def pieces_of(cs, first_small):
        """split a segment's columns into DMA pieces on CHUNK boundaries;
        a small first piece lets the first matmul start early"""
        out = []
        p0 = 0
        if first_small and cs > CHUNK:
            out.append((0, CHUNK))
            p0 = CHUNK
        while p0 < cs:
            pn = min(2 * CHUNK, cs - p0)
            out.append((p0, pn))
            p0 += pn
        return out

    with tile.TileContext(nc) as tc, ExitStack() as ctx:
        wpool = ctx.enter_context(tc.tile_pool(name="w", bufs=1))
        xpool = ctx.enter_context(tc.tile_pool(name="x", bufs=3))
        opool = ctx.enter_context(tc.tile_pool(name="o", bufs=4))
        pspool = ctx.enter_context(tc.tile_pool(name="ps", bufs=8, space="PSUM"))

        wt = wpool.tile([D_IN, N_SPECIES * N_OUT], MM_DT)

        n_copy = 0
        for s in range(N_SPECIES):
            cs = caps[s]
            if cs == 0:
                continue
            off = seg_off[s]
            # weights for this species (128 KB) just ahead of its x stream
            nc.sync.dma_start(wt[:, s * N_OUT:(s + 1) * N_OUT],
                              w[:, s * N_OUT:(s + 1) * N_OUT])
            # x segment in pieces so the first matmul starts early
            xt = xpool.tile([D_IN, cs], MM_DT, tag="x")
            pieces = pieces_of(cs, first_small=(s == 0))
            for (p0, pn) in pieces:
                nc.sync.dma_start(xt[:, p0:p0 + pn], xT[:, off + p0:off + p0 + pn])
            for h in range(2):
                lhsT = wt[:, s * N_OUT + h * 128: s * N_OUT + h * 128 + 128]
                ot = opool.tile([128, cs], F32, tag="o")
                for (p0, pn) in pieces:
                    for j0 in range(p0, p0 + pn, CHUNK):
                        cj = min(CHUNK, p0 + pn - j0)
                        ps = pspool.tile([128, CHUNK], F32, tag="ps")
                        nc.tensor.matmul(ps[:, :cj], lhsT, xt[:, j0:j0 + cj],
                                         start=True, stop=True)
                        if n_copy % 2 == 0:
                            nc.vector.tensor_copy(ot[:, j0:j0 + cj], ps[:, :cj])
                        else:
                            nc.scalar.copy(ot[:, j0:j0 + cj], ps[:, :cj])
                        n_copy += 1
                    # stream this piece of the output as soon as it's copied
                    nc.sync.dma_start(
                        outT[h * 128:(h + 1) * 128, off + p0:off + p0 + pn],
                        ot[:, p0:p0 + pn])

    nc.compile()
    return nc


def _prepare(values, species_idx, combining_matrix):
    """Host routing + packing. Returns (in_maps, plan)."""
    values = np.ascontiguousarray(values, dtype=np.float32)
    species_idx = np.asarray(species_idx, dtype=np.int32)
    w_host = np.ascontiguousarray(
        np.asarray(combining_matrix, dtype=np.float32).transpose(1, 0, 2).reshape(
            D_IN, N_SPECIES * N_OUT)
    )

    # per species, deal rows round-robin across cores (balanced +-1)
    core_rows = [[] for _ in range(N_CORES)]   # per core: list of row-index arrays
    counts = np.zeros((N_CORES, N_SPECIES), dtype=np.int64)
    for s in range(N_SPECIES):
        idx = np.nonzero(species_idx == s)[0]
        for c in range(N_CORES):
            sub = idx[c::N_CORES]
            core_rows[c].append(sub)
            counts[c, s] = sub.size

    caps = []
    for s in range(N_SPECIES):
        mx = int(counts[:, s].max())
        caps.append(0 if mx == 0 else -(-mx // PAD) * PAD)
    r_pad = int(sum(caps))
    offs = np.concatenate([[0], np.cumsum(caps)]).astype(np.int64)

    in_maps = []
    for c in range(N_CORES):
        xT = np.zeros((D_IN, r_pad), dtype=np.float32)
        for s in range(N_SPECIES):
            n = counts[c, s]
            if n:
                xT[:, offs[s]:offs[s] + n] = values[core_rows[c][s]].T
        in_maps.append({"xT": xT, "w": w_host})

    plan = {"core_rows": core_rows, "counts": counts, "caps": caps,
            "offs": offs, "r_pad": r_pad}
    return in_maps, plan


def _postprocess(results, plan):
    core_rows, counts, offs = plan["core_rows"], plan["counts"], plan["offs"]
    out = np.empty((M_TOTAL, N_OUT), dtype=np.float32)
    for c in range(N_CORES):
        oT = results[c]["outT"]
        for s in range(N_SPECIES):
            n = counts[c, s]
            if n:
                out[core_rows[c][s]] = oT[:, offs[s]:offs[s] + n].T
    return out


def kernel(values, species_idx, combining_matrix):
    in_maps, plan = _prepare(values, species_idx, combining_matrix)
    nc = _build_nc(plan["caps"], plan["r_pad"])
    res = run_bass_kernel_spmd(nc, in_maps, list(range(N_CORES)))
    return _postprocess(res.results, plan)


# revision 15
# speedup vs baseline: 1.1451x; 1.1451x over previous
"""Grouped-GEMM (MoE routing) kernel for TRN2, 8 NeuronCores, SPMD.

out[m] = values[m] @ combining_matrix[species_idx[m]]
  values [131072, 128] f32, species_idx [131072] i32, combining_matrix [8, 128, 256] f32

Strategy:
  - Host: counting-sort rows by species; deal each species' rows round-robin
    across the 8 cores so per-core per-species counts are balanced (+-1).
    Each core's rows are packed species-contiguous into a transposed buffer
    xT [128, R_pad] (species segment s zero-padded to a static capacity C[s],
    identical on every core -> one SPMD program).
  - Device (per core): keep all 8 weight matrices resident in SBUF
    ([128, 8*256] = 8KB/partition). For each species s and output half
    h in {0,1}: out_T[h*128:(h+1)*128, seg_s] = W[s][:, h*128:+128].T @ xT[:, seg_s]
    via matmuls with 512-column moving chunks (fp32, K=128 contraction on
    partitions). PSUM -> SBUF copy -> DMA to outT [256, R_pad].
  - Host: scatter outT columns back to the full [131072, 256] output.

This does 1x the FLOPs of the reference's 8x masked-matmul formulation and is
DMA-roofline-bound (~27 MB/core HBM traffic).
"""

import numpy as np
from contextlib import ExitStack

import concourse.bass as bass
import concourse.mybir as mybir
import concourse.tile as tile
from concourse import bacc
from concourse.bass_utils import run_bass_kernel_spmd

M_TOTAL = 131072
D_IN = 128
N_OUT = 256
N_SPECIES = 8
N_CORES = 8
PAD = 128          # species segment capacity granularity (rows)
CHUNK = 512        # matmul moving-dim chunk (max for fp32)
F32 = mybir.dt.float32
USE_FP32R = True   # fp32r matmul mode: full PE rate (vs 1/4 for fp32)
MM_DT = mybir.dt.float32r if USE_FP32R else mybir.dt.float32

def _build_nc(caps, r_pad):
    """Build the SPMD program for one core. caps[s] = padded column count of
    species segment s (same on all cores); r_pad = sum(caps)."""
    nc = bacc.Bacc("TRN2", target_bir_lowering=False, debug=False,
                   num_devices=N_CORES)
    xT = nc.dram_tensor("xT", [D_IN, r_pad], MM_DT, kind="ExternalInput").ap()
    w = nc.dram_tensor("w", [D_IN, N_SPECIES * N_OUT], MM_DT,
                       kind="ExternalInput").ap()
    outT = nc.dram_tensor("outT", [N_OUT, r_pad], F32, kind="ExternalOutput").ap()

    seg_off = {}
    off = 0
    for s in range(N_SPECIES):
        seg_off[s] = off
        off += caps[s]

    def pieces_of(cs, first_small):
        """split a segment's columns into DMA pieces on CHUNK boundaries;
        a small first piece lets the first matmul start early"""
        out = []
        p0 = 0
        if first_small and cs > CHUNK:
            out.append((0, CHUNK))
            p0 = CHUNK
        while p0 < cs:
            pn = min(2 * CHUNK, cs - p0)
            out.append((p0, pn))
            p0 += pn
        return out

    with tile.TileContext(nc) as tc, ExitStack() as ctx:
        wpool = ctx.enter_context(tc.tile_pool(name="w", bufs=1))
        xpool = ctx.enter_context(tc.tile_pool(name="x", bufs=3))
        opool = ctx.enter_context(tc.tile_pool(name="o", bufs=4))
        pspool = ctx.enter_context(tc.tile_pool(name="ps", bufs=8, space="PSUM"))

        wt = wpool.tile([D_IN, N_SPECIES * N_OUT], MM_DT)

        n_copy = 0
        for s in range(N_SPECIES):
            cs = caps[s]
            if cs == 0:
                continue
            off = seg_off[s]
            # weights for this species (128 KB) just ahead of its x stream
            nc.sync.dma_start(wt[:, s * N_OUT:(s + 1) * N_OUT],
                              w[:, s * N_OUT:(s + 1) * N_OUT])
            # x segment in pieces so the first matmul starts early
            xt = xpool.tile([D_IN, cs], MM_DT, tag="x")
            pieces = pieces_of(cs, first_small=(s == 0))
            for (p0, pn) in pieces:
                nc.sync.dma_start(xt[:, p0:p0 + pn], xT[:, off + p0:off + p0 + pn])
            for h in range(2):
                lhsT = wt[:, s * N_OUT + h * 128: s * N_OUT + h * 128 + 128]
                ot = opool.tile([128, cs], F32, tag="o")
                for (p0, pn) in pieces:
                    for j0 in range(p0, p0 + pn, CHUNK):
                        cj = min(CHUNK, p0 + pn - j0)
                        ps = pspool.tile([128, CHUNK], F32, tag="ps")
                        nc.tensor.matmul(ps[:, :cj], lhsT, xt[:, j0:j0 + cj],
                                         start=True, stop=True)
                        if n_copy % 2 == 0:
                            nc.vector.tensor_copy(ot[:, j0:j0 + cj], ps[:, :cj])
                        else:
                            nc.scalar.copy(ot[:, j0:j0 + cj], ps[:, :cj])
                        n_copy += 1
                    # stream this piece of the output as soon as it's copied
                    nc.gpsimd.dma_start(
                        outT[h * 128:(h + 1) * 128, off + p0:off + p0 + pn],
                        ot[:, p0:p0 + pn])

    nc.compile()
    return nc


def _prepare(values, species_idx, combining_matrix):
    """Host routing + packing. Returns (in_maps, plan)."""
    values = np.ascontiguousarray(values, dtype=np.float32)
    species_idx = np.asarray(species_idx, dtype=np.int32)
    w_host = np.ascontiguousarray(
        np.asarray(combining_matrix, dtype=np.float32).transpose(1, 0, 2).reshape(
            D_IN, N_SPECIES * N_OUT)
    )

    # per species, deal rows round-robin across cores (balanced +-1)
    core_rows = [[] for _ in range(N_CORES)]   # per core: list of row-index arrays
    counts = np.zeros((N_CORES, N_SPECIES), dtype=np.int64)
    for s in range(N_SPECIES):
        idx = np.nonzero(species_idx == s)[0]
        for c in range(N_CORES):
            sub = idx[c::N_CORES]
            core_rows[c].append(sub)
            counts[c, s] = sub.size

    caps = []
    for s in range(N_SPECIES):
        mx = int(counts[:, s].max())
        caps.append(0 if mx == 0 else -(-mx // PAD) * PAD)
    r_pad = int(sum(caps))
    offs = np.concatenate([[0], np.cumsum(caps)]).astype(np.int64)

    in_maps = []
    for c in range(N_CORES):
        xT = np.zeros((D_IN, r_pad), dtype=np.float32)
        for s in range(N_SPECIES):
            n = counts[c, s]
            if n:
                xT[:, offs[s]:offs[s] + n] = values[core_rows[c][s]].T
        in_maps.append({"xT": xT, "w": w_host})

    plan = {"core_rows": core_rows, "counts": counts, "caps": caps,
            "offs": offs, "r_pad": r_pad}
    return in_maps, plan


def _postprocess(results, plan):
    core_rows, counts, offs = plan["core_rows"], plan["counts"], plan["offs"]
    out = np.empty((M_TOTAL, N_OUT), dtype=np.float32)
    for c in range(N_CORES):
        oT = results[c]["outT"]
        for s in range(N_SPECIES):
            n = counts[c, s]
            if n:
                out[core_rows[c][s]] = oT[:, offs[s]:offs[s] + n].T
    return out


def kernel(values, species_idx, combining_matrix):
    in_maps, plan = _prepare(values, species_idx, combining_matrix)
    nc = _build_nc(plan["caps"], plan["r_pad"])
    res = run_bass_kernel_spmd(nc, in_maps, list(range(N_CORES)))
    return _postprocess(res.results, plan)


# revision 17
# speedup vs baseline: 1.1852x; 1.0350x over previous
"""Grouped-GEMM (MoE routing) kernel for TRN2, 8 NeuronCores, SPMD.

out[m] = values[m] @ combining_matrix[species_idx[m]]
  values [131072, 128] f32, species_idx [131072] i32, combining_matrix [8, 128, 256] f32

Strategy:
  - Host: counting-sort rows by species; deal each species' rows round-robin
    across the 8 cores so per-core per-species counts are balanced (+-1).
    Each core's rows are packed species-contiguous into a transposed buffer
    xT [128, R_pad] (species segment s zero-padded to a static capacity C[s],
    identical on every core -> one SPMD program).
  - Device (per core): keep all 8 weight matrices resident in SBUF
    ([128, 8*256] = 8KB/partition). For each species s and output half
    h in {0,1}: out_T[h*128:(h+1)*128, seg_s] = W[s][:, h*128:+128].T @ xT[:, seg_s]
    via matmuls with 512-column moving chunks (fp32, K=128 contraction on
    partitions). PSUM -> SBUF copy -> DMA to outT [256, R_pad].
  - Host: scatter outT columns back to the full [131072, 256] output.

This does 1x the FLOPs of the reference's 8x masked-matmul formulation and is
DMA-roofline-bound (~27 MB/core HBM traffic).
"""

import numpy as np
from contextlib import ExitStack

import concourse.bass as bass
import concourse.mybir as mybir
import concourse.tile as tile
from concourse import bacc
from concourse.bass_utils import run_bass_kernel_spmd

M_TOTAL = 131072
D_IN = 128
N_OUT = 256
N_SPECIES = 8
N_CORES = 8
PAD = 128          # species segment capacity granularity (rows)
CHUNK = 512        # matmul moving-dim chunk (max for fp32)
F32 = mybir.dt.float32
USE_FP32R = True   # fp32r matmul mode: full PE rate (vs 1/4 for fp32)
MM_DT = mybir.dt.float32r if USE_FP32R else mybir.dt.float32

def _build_nc(caps, r_pad):
    """Build the SPMD program for one core. caps[s] = padded column count of
    species segment s (same on all cores); r_pad = sum(caps)."""
    nc = bacc.Bacc("TRN2", target_bir_lowering=False, debug=False,
                   num_devices=N_CORES)
    xT = nc.dram_tensor("xT", [D_IN, r_pad], MM_DT, kind="ExternalInput").ap()
    w = nc.dram_tensor("w", [D_IN, N_SPECIES * N_OUT], MM_DT,
                       kind="ExternalInput").ap()
    outT = nc.dram_tensor("outT", [N_OUT, r_pad], F32, kind="ExternalOutput").ap()

    seg_off = {}
    off = 0
    for s in range(N_SPECIES):
        seg_off[s] = off
        off += caps[s]

    def pieces_of(cs, first_small):
        """split a segment's columns into DMA pieces on CHUNK boundaries;
        a small first piece lets the first matmul start early"""
        out = []
        p0 = 0
        if first_small and cs > CHUNK:
            out.append((0, CHUNK))
            p0 = CHUNK
        while p0 < cs:
            pn = min(2 * CHUNK, cs - p0)
            out.append((p0, pn))
            p0 += pn
        return out

    with tile.TileContext(nc) as tc, ExitStack() as ctx:
        wpool = ctx.enter_context(tc.tile_pool(name="w", bufs=1))
        xpool = ctx.enter_context(tc.tile_pool(name="x", bufs=3))
        opool = ctx.enter_context(tc.tile_pool(name="o", bufs=4))
        pspool = ctx.enter_context(tc.tile_pool(name="ps", bufs=8, space="PSUM"))

        wt = wpool.tile([D_IN, N_SPECIES * N_OUT], MM_DT)

        n_copy = 0
        for s in range(N_SPECIES):
            cs = caps[s]
            if cs == 0:
                continue
            off = seg_off[s]
            # weights for this species (128 KB) just ahead of its x stream
            nc.sync.dma_start(wt[:, s * N_OUT:(s + 1) * N_OUT],
                              w[:, s * N_OUT:(s + 1) * N_OUT])
            # x segment in pieces so the first matmul starts early
            xt = xpool.tile([D_IN, cs], MM_DT, tag="x")
            pieces = pieces_of(cs, first_small=(s == 0))
            for (p0, pn) in pieces:
                nc.sync.dma_start(xt[:, p0:p0 + pn], xT[:, off + p0:off + p0 + pn])
            for h in range(2):
                lhsT = wt[:, s * N_OUT + h * 128: s * N_OUT + h * 128 + 128]
                ot = opool.tile([128, cs], F32, tag="o")
                for (p0, pn) in pieces:
                    for j0 in range(p0, p0 + pn, CHUNK):
                        cj = min(CHUNK, p0 + pn - j0)
                        ps = pspool.tile([128, CHUNK], F32, tag="ps")
                        nc.tensor.matmul(ps[:, :cj], lhsT, xt[:, j0:j0 + cj],
                                         start=True, stop=True)
                        if n_copy % 2 == 0:
                            nc.vector.tensor_copy(ot[:, j0:j0 + cj], ps[:, :cj])
                        else:
                            nc.scalar.copy(ot[:, j0:j0 + cj], ps[:, :cj])
                        n_copy += 1
                    # stream this piece of the output as soon as it's copied
                    nc.gpsimd.dma_start(
                        outT[h * 128:(h + 1) * 128, off + p0:off + p0 + pn],
                        ot[:, p0:p0 + pn])

    nc.compile()
    return nc


def _prepare(values, species_idx, combining_matrix):
    """Host routing + packing. Returns (in_maps, plan)."""
    values = np.ascontiguousarray(values, dtype=np.float32)
    species_idx = np.asarray(species_idx, dtype=np.int32)
    w_host = np.ascontiguousarray(
        np.asarray(combining_matrix, dtype=np.float32).transpose(1, 0, 2).reshape(
            D_IN, N_SPECIES * N_OUT)
    )

    # per species, deal rows round-robin across cores (balanced +-1)
    core_rows = [[] for _ in range(N_CORES)]   # per core: list of row-index arrays
    counts = np.zeros((N_CORES, N_SPECIES), dtype=np.int64)
    for s in range(N_SPECIES):
        idx = np.nonzero(species_idx == s)[0]
        for c in range(N_CORES):
            sub = idx[c::N_CORES]
            core_rows[c].append(sub)
            counts[c, s] = sub.size

    caps = []
    for s in range(N_SPECIES):
        mx = int(counts[:, s].max())
        caps.append(0 if mx == 0 else -(-mx // PAD) * PAD)
    r_pad = int(sum(caps))
    offs = np.concatenate([[0], np.cumsum(caps)]).astype(np.int64)

    in_maps = []
    for c in range(N_CORES):
        xT = np.zeros((D_IN, r_pad), dtype=np.float32)
        for s in range(N_SPECIES):
            n = counts[c, s]
            if n:
                xT[:, offs[s]:offs[s] + n] = values[core_rows[c][s]].T
        in_maps.append({"xT": xT, "w": w_host})

    plan = {"core_rows": core_rows, "counts": counts, "caps": caps,
            "offs": offs, "r_pad": r_pad}
    return in_maps, plan


def _postprocess(results, plan):
    core_rows, counts, offs = plan["core_rows"], plan["counts"], plan["offs"]
    out = np.empty((M_TOTAL, N_OUT), dtype=np.float32)
    for c in range(N_CORES):
        oT = results[c]["outT"]
        for s in range(N_SPECIES):
            n = counts[c, s]
            if n:
                out[core_rows[c][s]] = oT[:, offs[s]:offs[s] + n].T
    return out


def kernel(values, species_idx, combining_matrix):
    in_maps, plan = _prepare(values, species_idx, combining_matrix)
    nc = _build_nc(plan["caps"], plan["r_pad"])
    res = run_bass_kernel_spmd(nc, in_maps, list(range(N_CORES)))
    return _postprocess(res.results, plan)
